# revision 1
# baseline (speedup 1.0000x reference)
"""Binarized MLP (784 -> 1024 -> 1024 -> 1024 -> 10) on 8 TRN2 NeuronCores.

Data-parallel over the batch (16384 rows -> 2048 per core), weights replicated.

Math notes (these make the kernel both fast and numerically faithful):
  * Layers 1-2 outputs are only ever consumed through binarize(hardtanh(bn(h))).
    Since hardtanh preserves sign and bn here is (h - m) * rsqrt(v+eps) * g + be
    with g > 0, be == 0, the next-layer input is exactly sign(h + (b - m)).
    That is one ScalarE Sign activation with a per-partition bias, no bn needed.
  * fc2/fc3 multiply two +-1 operands -> exact in fp8(e4m3) with fp32 PSUM
    accumulation (integer partial sums, magnitude <= 1024). DoubleRow perf mode
    contracts two 128-row chunks per pass (2 fp8 weights per PE cell).
  * fc1 keeps x at full precision via an exact fp16 hi/lo split:
    x = hi + lo with hi = fp16(x), lo = fp16(x - hi); products with +-1 weights
    are exact, so accuracy ~ fp32 matmul, at 2 bf16-rate passes.
  * fc4 + log_softmax: logits computed feature-major [10, B], PE-transposed to
    [B, 10]; log_softmax without max-subtraction (logits are small; exp is safe).

Loop order: weights stationary per (m, k); all 4 batch column chunks stream
per weight load (amortizes LDWEIGHTS). 4 PSUM banks accumulate per m-tile,
8-slot pool double-buffers across m-tiles.
"""

import os
import numpy as np

N_CORES = 8
B_FULL = 16384
BS = B_FULL // N_CORES  # 2048 rows per core
IN_F = 784
K1C = 7                 # s1 weight chunks of 128 (784 padded to 896; chunk 6 = packed tail)
XC = 13                 # fc1 x chunks: 6 hi + 6 lo + 1 packed hi/lo tail
H = 1024
HC = 8                  # hidden chunks of 128
OUT_F = 10
NSPLIT = 4              # batch column chunks of 512
NB = BS // NSPLIT       # 512
BT = BS // 128          # 16 batch tiles of 128 for the output transpose

LAST_RESULT = None      # BassKernelResults of the most recent run (for test.py)

_PLAN = {}


def _build_nc():
    import concourse.bass as bass
    import concourse.mybir as mybir
    import concourse.tile as tile
    from concourse.tile import add_dep_helper
    from concourse import bacc
    from concourse.bass import ts
    from concourse.masks import make_identity

    f32 = mybir.dt.float32
    f16 = mybir.dt.float16
    f8 = mybir.dt.float8e4
    AF = mybir.ActivationFunctionType
    ALU = mybir.AluOpType
    DR = mybir.MatmulPerfMode.DoubleRow

    nc = bacc.Bacc(None)

    x_t = nc.dram_tensor("xc", [XC, 128, BS], f16, kind="ExternalInput")
    s1_t = nc.dram_tensor("s1t", [HC, K1C, 128, 128], f16, kind="ExternalInput")
    s2_t = nc.dram_tensor("s2t", [HC, HC, 128, 128], f8, kind="ExternalInput")
    s3_t = nc.dram_tensor("s3t", [HC, HC, 128, 128], f8, kind="ExternalInput")
    w4_t = nc.dram_tensor("w4t", [HC, 128, OUT_F], f16, kind="ExternalInput")
    b1_t = nc.dram_tensor("bias1", [H], f32, kind="ExternalInput")
    b2_t = nc.dram_tensor("bias2", [H], f32, kind="ExternalInput")
    sc3_t = nc.dram_tensor("sc3", [H], f32, kind="ExternalInput")
    sh3_t = nc.dram_tensor("sh3", [H], f32, kind="ExternalInput")
    b4_t = nc.dram_tensor("b4", [OUT_F], f32, kind="ExternalInput")
    y_t = nc.dram_tensor("y", [BS, OUT_F], f32, kind="ExternalOutput")

    with tile.TileContext(nc) as tc:
        with (
            tc.tile_pool(name="consts", bufs=1) as consts,
            tc.tile_pool(name="tmp", bufs=4) as tmp,
            tc.tile_pool(name="psum", bufs=8, space="PSUM") as psum,
        ):
            x_sb = consts.tile([128, XC, BS], f16, tag="xc")
            s1_sb = consts.tile([128, HC, K1C, 128], f16, tag="s1")
            s2_sb = consts.tile([128, HC, HC, 128], f8, tag="s2")
            s3_sb = consts.tile([128, HC, HC, 128], f8, tag="s3")
            w4_sb = consts.tile([128, HC, OUT_F], f16, tag="w4")
            b1v = consts.tile([128, HC], f32, tag="b1v")
            b2v = consts.tile([128, HC], f32, tag="b2v")
            sc3v = consts.tile([128, HC], f32, tag="sc3v")
            sh3v = consts.tile([128, HC], f32, tag="sh3v")
            b4bc = consts.tile([128, OUT_F], f32, tag="b4bc")
            act1 = consts.tile([128, HC, BS], f8, tag="act1")
            act2 = consts.tile([128, HC, BS], f8, tag="act2")
            act3 = consts.tile([128, HC, BS], f16, tag="act3")
            NBLK = BS // 32  # 64 batch blocks of 32 for the DVE transpose
            logits = consts.tile([32, BS], f32, tag="logits")
            ltr = consts.tile([32, BS], f32, tag="ltr")
            es2 = consts.tile([32, NBLK, OUT_F], f32, tag="es2")
            lse2 = consts.tile([32, NBLK], f32, tag="lse2")
            outf2 = consts.tile([32, NBLK, OUT_F], f32, tag="outf2")

            # ---- input DMAs. First-needed pieces are split fine so the
            # PE can start as soon as possible; x rides the two HWDGE
            # rings, alternating (per-ring FIFO keeps chunk completion in
            # consumption order at full bandwidth); s1 m-tiles staggered.
            nc.gpsimd.dma_start(
                out=s1_sb[:, 0, 0:2], in_=s1_t[0, 0:2].rearrange("k p c -> p k c")
            )
            for n in range(NSPLIT):
                nc.sync.dma_start(out=x_sb[:, 0, ts(n, NB)], in_=x_t[0, :, ts(n, NB)])
            nc.scalar.dma_start(
                out=s1_sb[:, 0, 2:K1C], in_=s1_t[0, 2:K1C].rearrange("k p c -> p k c")
            )
            nc.gpsimd.dma_start(out=b1v, in_=b1_t[:].rearrange("(m p) -> p m", p=128))
            dma_engs = [nc.sync, nc.scalar]
            for k in range(1, XC):
                dma_engs[k % 2].dma_start(out=x_sb[:, k], in_=x_t[k])
            s1_dmas = {}
            for m in range(1, HC):
                s1_dmas[m] = nc.gpsimd.dma_start(
                    out=s1_sb[:, m], in_=s1_t[m].rearrange("k p c -> p k c")
                )

            # ---- fc1: h1 = xT.T @ s1T (feature-major), sign -> act1 ----
            # x chunks: 0-5 = hi rows 0-767, 6-11 = lo rows 0-767,
            # 12 = packed tail (hi rows 768-783 @p0-15, lo @p32-47).
            for m in range(HC):
                pss = [psum.tile([128, NB], f32, tag="mm", name="ps") for _ in range(NSPLIT)]
                for k in range(XC):
                    wk = k if k < 6 else (k - 6 if k < 12 else 6)
                    for n in range(NSPLIT):
                        nc.tensor.matmul(
                            pss[n], s1_sb[:, m, wk], x_sb[:, k, ts(n, NB)],
                            start=(k == 0), stop=(k == XC - 1),
                        )
                for n in range(NSPLIT):
                    a = nc.scalar.activation(
                        act1[:, m, ts(n, NB)], pss[n], AF.Sign, bias=b1v[:, m:m + 1]
                    )
                    if m == 1 and n == NSPLIT - 1:
                        x_done_gate = a
                    # stagger s1 weight loads two m-tiles ahead of use
                    if n == 0 and m + 3 in s1_dmas:
                        add_dep_helper(s1_dmas[m + 3].ins, a.ins,
                                       reason="stagger s1 loads")

            # later-layer weights: gated behind fc1 m=1 so their transfers
            # don't steal HBM bandwidth from the x load during the ramp
            for m in range(HC):
                d = nc.gpsimd.dma_start(
                    out=s2_sb[:, m], in_=s2_t[m].rearrange("k p c -> p k c")
                )
                add_dep_helper(d.ins, x_done_gate.ins, reason="defer s2 after x load")
            nc.sync.dma_start(out=b2v, in_=b2_t[:].rearrange("(m p) -> p m", p=128))
            for m in range(HC):
                d = nc.gpsimd.dma_start(
                    out=s3_sb[:, m], in_=s3_t[m].rearrange("k p c -> p k c")
                )
                add_dep_helper(d.ins, x_done_gate.ins, reason="defer s3 after x load")
            nc.sync.dma_start(out=sc3v, in_=sc3_t[:].rearrange("(m p) -> p m", p=128))
            nc.sync.dma_start(out=sh3v, in_=sh3_t[:].rearrange("(m p) -> p m", p=128))
            nc.sync.dma_start(out=w4_sb, in_=w4_t.rearrange("k p o -> p k o"))
            b4_ap = b4_t[:]
            nc.sync.dma_start(
                out=b4bc,
                in_=bass.AP(tensor=b4_ap.tensor, offset=b4_ap.offset,
                            ap=[[0, 128]] + list(b4_ap.ap)),
            )
            nc.vector.memset(logits, 0.0)

            # ---- fc2: binary x binary, fp8 DoubleRow, sign -> act2 ----
            for m in range(HC):
                pss = [psum.tile([128, NB], f32, tag="mm", name="ps") for _ in range(NSPLIT)]
                for kk in range(HC // 2):
                    ksl = slice(2 * kk, 2 * kk + 2)
                    for n in range(NSPLIT):
                        nc.tensor.matmul(
                            pss[n], s2_sb[:, m, ksl], act1[:, ksl, ts(n, NB)],
                            start=(kk == 0), stop=(kk == HC // 2 - 1),
                            perf_mode=DR,
                        )
                for n in range(NSPLIT):
                    nc.scalar.activation(
                        act2[:, m, ts(n, NB)], pss[n], AF.Sign, bias=b2v[:, m:m + 1]
                    )

            # ---- fc3: fp8 DoubleRow, bn affine + hardtanh -> act3 (DVE) ----
            for m in range(HC):
                pss = [psum.tile([128, NB], f32, tag="mm", name="ps") for _ in range(NSPLIT)]
                for kk in range(HC // 2):
                    ksl = slice(2 * kk, 2 * kk + 2)
                    for n in range(NSPLIT):
                        nc.tensor.matmul(
                            pss[n], s3_sb[:, m, ksl], act2[:, ksl, ts(n, NB)],
                            start=(kk == 0), stop=(kk == HC // 2 - 1),
                            perf_mode=DR,
                        )
                for n in range(NSPLIT):
                    t = tmp.tile([128, NB], f32, tag="t3")
                    nc.scalar.activation(
                        t, pss[n], AF.Identity,
                        bias=sh3v[:, m:m + 1], scale=sc3v[:, m:m + 1],
                    )
                    nc.vector.tensor_scalar(
                        out=act3[:, m, ts(n, NB)], in0=t,
                        scalar1=-1.0, scalar2=1.0,
                        op0=ALU.max, op1=ALU.min,
                    )

            # ---- fc4: logits[10, BS]; DVE 32x32 block transpose per n ----
            # ltr[p, 32j+q] = logit class q of batch row 32j+p  (q < 10)
            for n in range(NSPLIT):
                ps4 = psum.tile([OUT_F, NB], f32, tag="mm", name="ps4")
                for k in range(HC):
                    nc.tensor.matmul(
                        ps4, w4_sb[:, k], act3[:, k, ts(n, NB)],
                        start=(k == 0), stop=(k == HC - 1),
                    )
                nc.scalar.copy(logits[0:OUT_F, ts(n, NB)], ps4)
                nc.vector.transpose(ltr[:, ts(n, NB)], logits[:, ts(n, NB)])
            ltv = bass.AP(tensor=ltr[:].tensor, offset=ltr[:].offset,
                          ap=[ltr[:].ap[0], [32, NBLK], [1, OUT_F]])
            b4r = b4bc[:]
            nc.vector.tensor_tensor(
                out=ltv, in0=ltv,
                in1=bass.AP(tensor=b4r.tensor, offset=b4r.offset,
                            ap=[[b4r.ap[0][0], 32], [0, NBLK], b4r.ap[1]]),
                op=ALU.add,
            )
            nc.scalar.activation(es2, ltv, AF.Exp)
            nc.vector.tensor_reduce(
                out=lse2, in_=es2, axis=mybir.AxisListType.X, op=ALU.add
            )
            nc.scalar.activation(lse2, lse2, AF.Ln)
            lser = lse2[:]
            nc.vector.tensor_tensor(
                out=outf2, in0=ltv,
                in1=bass.AP(tensor=lser.tensor, offset=lser.offset,
                            ap=[lser.ap[0], lser.ap[1], [0, OUT_F]]),
                op=ALU.subtract,
            )
            nc.sync.dma_start(
                out=y_t.rearrange("(j p) o -> p j o", p=32), in_=outf2
            )

    nc.finalize()
    return nc


def _host_prep(inputs):
    """Shard x, binarize/lay out weights, fold bn into sign biases."""
    import ml_dtypes

    f16 = np.float16
    f8 = ml_dtypes.float8_e4m3

    x = np.asarray(inputs["x"], np.float32)
    w1 = np.asarray(inputs["w1"], np.float32)
    w2 = np.asarray(inputs["w2"], np.float32)
    w3 = np.asarray(inputs["w3"], np.float32)
    w4 = np.asarray(inputs["w4"], np.float32)
    b1 = np.asarray(inputs["b1"], np.float32)
    b2 = np.asarray(inputs["b2"], np.float32)
    b3 = np.asarray(inputs["b3"], np.float32)
    b4 = np.asarray(inputs["b4"], np.float32)

    EPS = np.float64(1e-5)

    def gv(i):
        return (np.asarray(inputs[f"g{i}"], np.float32),
                np.asarray(inputs[f"be{i}"], np.float32),
                np.asarray(inputs[f"m{i}"], np.float32),
                np.asarray(inputs[f"v{i}"], np.float32))

    g1, be1, m1, v1 = gv(1)
    g2, be2, m2, v2 = gv(2)
    g3, be3, m3, v3 = gv(3)
    # sign(bn(h)) == sign(h + (b - m)) requires gamma > 0 and beta == 0
    assert np.all(g1 > 0) and np.all(be1 == 0), "unsupported bn1 params"
    assert np.all(g2 > 0) and np.all(be2 == 0), "unsupported bn2 params"

    bias1 = (b1 - m1).astype(np.float32)
    bias2 = (b2 - m2).astype(np.float32)
    r3 = 1.0 / np.sqrt(v3.astype(np.float64) + EPS)
    sc3 = (r3 * g3).astype(np.float32)
    sh3 = ((b3 - m3).astype(np.float64) * r3 * g3 + be3).astype(np.float32)

    def wlay(w, kc, dt):  # [out, in] -> [m, k, 128p(in), 128c(out)]
        st = np.sign(w).T.astype(np.float32)            # [in, out]
        kin = kc * 128
        if st.shape[0] < kin:
            st = np.pad(st, ((0, kin - st.shape[0]), (0, 0)))
        mo = st.shape[1] // 128
        return np.ascontiguousarray(
            st.reshape(kc, 128, mo, 128).transpose(2, 0, 1, 3)
        ).astype(dt)

    # s1: chunks 0-5 = rows 0-767; chunk 6 = packed tail (rows 768-783
    # replicated at partitions 0-15 and 32-47, matching the x tail chunk)
    s1f = np.sign(w1).T.astype(np.float32)              # [784, 1024]
    s1t = np.zeros((HC, K1C, 128, 128), np.float32)
    body = s1f[:768].reshape(6, 128, HC, 128)
    tail = s1f[768:784].reshape(16, HC, 128)
    for m in range(HC):
        s1t[m, :6] = body[:, :, m]
        s1t[m, 6, 0:16] = tail[:, m]
        s1t[m, 6, 32:48] = tail[:, m]
    s1t = s1t.astype(f16)
    s2t = wlay(w2, HC, f8)
    s3t = wlay(w3, HC, f8)
    w4t = np.ascontiguousarray(w4.T.astype(f16)).reshape(HC, 128, OUT_F)

    shared = dict(s1t=s1t, s2t=s2t, s3t=s3t, w4t=w4t,
                  bias1=bias1, bias2=bias2, sc3=sc3, sh3=sh3, b4=b4)
    in_maps = []
    for c in range(N_CORES):
        xs = x[c * BS:(c + 1) * BS]                     # [2048, 784]
        xt = xs.T                                       # [784, 2048]
        xhi = xt.astype(f16)
        xlo = (xt - xhi.astype(np.float32)).astype(f16)
        xc = np.zeros((XC, 128, BS), f16)
        xc[0:6] = xhi[:768].reshape(6, 128, BS)
        xc[6:12] = xlo[:768].reshape(6, 128, BS)
        xc[12, 0:16] = xhi[768:784]
        xc[12, 32:48] = xlo[768:784]
        m = dict(shared)
        m["xc"] = np.ascontiguousarray(xc)
        in_maps.append(m)
    return in_maps


def kernel(**inputs):
    global LAST_RESULT
    from concourse.bass_utils import run_bass_kernel_spmd

    if "nc" not in _PLAN:
        _PLAN["nc"] = _build_nc()
    nc = _PLAN["nc"]

    in_maps = _host_prep(inputs)
    br = run_bass_kernel_spmd(
        nc, in_maps, list(range(N_CORES)),
        tmpdir=os.environ.get("KERNEL_TMPDIR") or None,
    )
    LAST_RESULT = br
    out = np.concatenate([br.results[c]["y"] for c in range(N_CORES)], axis=0)
    return out.astype(np.float32)



# revision 2
# speedup vs baseline: 1.0839x; 1.0839x over previous
"""Binarized MLP (784 -> 1024 -> 1024 -> 1024 -> 10) on 8 TRN2 NeuronCores.

Data-parallel over the batch (16384 rows -> 2048 per core), weights replicated.

Math notes:
  * Layers 1-2 outputs are only consumed through binarize(hardtanh(bn(h))).
    With g > 0, be == 0 that is exactly sign(h + (b - m)): one ScalarE Sign
    activation with a per-partition bias.
  * fc2/fc3 multiply two +-1 operands -> exact in fp8(e4m3) with fp32 PSUM
    accumulation; DoubleRow contracts 256 rows per 512-cycle pass.
  * fc1 keeps x near-full precision in 10 PE passes (vs 13 for exact hi/lo):
    rows 0-767 as f16 hi (6 passes) plus an fp8 residual lo = x - f16(x)
    scaled by 2^12 contracted in 3 fp8-DR passes with weights sign*2^-6;
    the whole layer runs at a 2^6 global scale (hi weights sign*64, bias*64)
    so every fp8 value stays in the normal range. Sign() is scale-invariant.
    Rows 768-783 ride one packed K=32 f16 pass (hi rows at p0-15, f16 lo at
    p16-31) which is hi/lo-exact. Host-simulated rel err of this scheme is
    9.4e-3 (22 sign flips of 16.7M act1 bits), comfortably inside 2e-2.
  * fc1 loops n-outer (batch 512-col blocks) so the first block only needs
    1/4 of x before the PE can stream - kills the DMA-starved ramp.
  * Batch columns are host-permuted (col c carries batch row 64*(c%32)+c//32)
    so after the DVE 32x32 output transpose each partition holds contiguous
    y rows: the output DMA becomes 32 x 640B runs instead of 2048 x 40B.
  * fc4 + log_softmax run per 512-col block, overlapping later fc4 matmuls;
    dummy Sign/Exp activations early pull both ACT table loads into the ramp.
"""

import os
import numpy as np

N_CORES = 8
B_FULL = 16384
BS = B_FULL // N_CORES  # 2048 rows per core
IN_F = 784
H = 1024
HC = 8                  # hidden chunks of 128
OUT_F = 10
NSPLIT = 4              # batch column blocks of 512
NB = BS // NSPLIT       # 512
KH = 6                  # f16 hi chunks (rows 0-767)
KL = 6                  # fp8 lo chunks = 3 DoubleRow pairs (rows 0-767)
NJ = NB // 32           # 16 32-col blocks per n-block (output transpose)

LAST_RESULT = None      # BassKernelResults of the most recent run (for test.py)

_PLAN = {}


def _build_nc():
    import concourse.bass as bass
    import concourse.mybir as mybir
    import concourse.tile as tile
    from concourse.tile import add_dep_helper
    from concourse import bacc
    from concourse.bass import ts

    f32 = mybir.dt.float32
    f16 = mybir.dt.float16
    f8 = mybir.dt.float8e4
    AF = mybir.ActivationFunctionType
    ALU = mybir.AluOpType
    DR = mybir.MatmulPerfMode.DoubleRow

    nc = bacc.Bacc(None)

    xh_t = nc.dram_tensor("xh", [NSPLIT, KH, 128, NB], f16, kind="ExternalInput")
    xt_t = nc.dram_tensor("xt", [NSPLIT, 32, NB], f16, kind="ExternalInput")
    xl_t = nc.dram_tensor("xl", [NSPLIT, KL, 128, NB], f8, kind="ExternalInput")
    s1h_t = nc.dram_tensor("s1h", [HC, KH, 128, 128], f16, kind="ExternalInput")
    s1t_t = nc.dram_tensor("s1t", [HC, 32, 128], f16, kind="ExternalInput")
    s1l_t = nc.dram_tensor("s1l", [HC, KL, 128, 128], f8, kind="ExternalInput")
    s2_t = nc.dram_tensor("s2t", [HC, HC, 128, 128], f8, kind="ExternalInput")
    s3_t = nc.dram_tensor("s3t", [HC, HC, 128, 128], f8, kind="ExternalInput")
    w4_t = nc.dram_tensor("w4t", [HC, 128, OUT_F], f16, kind="ExternalInput")
    b1_t = nc.dram_tensor("bias1", [H], f32, kind="ExternalInput")
    b2_t = nc.dram_tensor("bias2", [H], f32, kind="ExternalInput")
    sc3_t = nc.dram_tensor("sc3", [H], f32, kind="ExternalInput")
    sh3_t = nc.dram_tensor("sh3", [H], f32, kind="ExternalInput")
    b4_t = nc.dram_tensor("b4", [OUT_F], f32, kind="ExternalInput")
    y_t = nc.dram_tensor("y", [BS, OUT_F], f32, kind="ExternalOutput")

    with tile.TileContext(nc) as tc:
        with (
            tc.tile_pool(name="consts", bufs=1) as consts,
            tc.tile_pool(name="tmp", bufs=4) as tmp,
            tc.tile_pool(name="psum", bufs=8, space="PSUM") as psum,
        ):
            xh_sb = consts.tile([128, NSPLIT, KH, NB], f16, tag="xh")
            xt_sb = consts.tile([32, NSPLIT, NB], f16, tag="xt")
            xl_sb = consts.tile([128, NSPLIT, KL, NB], f8, tag="xl")
            s1h_sb = consts.tile([128, HC, KH, 128], f16, tag="s1h")
            s1t_sb = consts.tile([32, HC, 128], f16, tag="s1t")
            s1l_sb = consts.tile([128, HC, KL, 128], f8, tag="s1l")
            s2_sb = consts.tile([128, HC, HC, 128], f8, tag="s2")
            s3_sb = consts.tile([128, HC, HC, 128], f8, tag="s3")
            w4_sb = consts.tile([128, HC, OUT_F], f16, tag="w4")
            b1v = consts.tile([128, HC], f32, tag="b1v")
            b2v = consts.tile([128, HC], f32, tag="b2v")
            sc3v = consts.tile([128, HC], f32, tag="sc3v")
            sh3v = consts.tile([128, HC], f32, tag="sh3v")
            b4bc = consts.tile([32, OUT_F], f32, tag="b4bc")
            act1 = consts.tile([128, HC, BS], f8, tag="act1")
            act2 = consts.tile([128, HC, BS], f8, tag="act2")
            act3 = consts.tile([128, HC, BS], f16, tag="act3")
            logits = consts.tile([32, BS], f32, tag="logits")
            ltr = consts.tile([32, BS], f32, tag="ltr")
            es = consts.tile([32, NSPLIT * NJ, OUT_F], f32, tag="es")
            lse = consts.tile([32, NSPLIT * NJ], f32, tag="lse")
            outf = consts.tile([32, NSPLIT * NJ, OUT_F], f32, tag="outf")
            dum = consts.tile([1, 2], f32, tag="dum")

            # ---- ramp DMAs. The first n-block (hi, tail, lo) plus m=0
            # weights land first; the two HWDGE rings (sync/scalar) carry x,
            # gpsimd carries weights.
            nc.gpsimd.dma_start(
                out=s1h_sb[:, 0], in_=s1h_t[0].rearrange("k p c -> p k c")
            )
            nc.sync.dma_start(
                out=xh_sb[:, 0, 0:3], in_=xh_t[0, 0:3].rearrange("k p c -> p k c")
            )
            nc.scalar.dma_start(
                out=xh_sb[:, 0, 3:KH], in_=xh_t[0, 3:KH].rearrange("k p c -> p k c")
            )
            nc.gpsimd.dma_start(out=b1v, in_=b1_t[:].rearrange("(m p) -> p m", p=128))
            nc.gpsimd.dma_start(out=s1t_sb, in_=s1t_t.rearrange("m p c -> p m c"))
            nc.gpsimd.dma_start(
                out=s1l_sb[:, 0], in_=s1l_t[0].rearrange("k p c -> p k c")
            )
            nc.sync.dma_start(
                out=xl_sb[:, 0], in_=xl_t[0].rearrange("k p c -> p k c")
            )
            nc.scalar.dma_start(out=xt_sb, in_=xt_t.rearrange("n p c -> p n c"))
            # preload both activation tables while ScalarE is otherwise idle
            nc.vector.memset(dum, 0.0)
            nc.scalar.activation(dum, dum, AF.Sign)
            nc.scalar.activation(dum, dum, AF.Exp)
            nc.vector.memset(logits, 0.0)
            # remaining fc1 weights (m 1-3 then 4-7, hi before lo)
            nc.gpsimd.dma_start(
                out=s1h_sb[:, 1:4], in_=s1h_t[1:4].rearrange("m k p c -> p m k c")
            )
            nc.gpsimd.dma_start(
                out=s1l_sb[:, 1:4], in_=s1l_t[1:4].rearrange("m k p c -> p m k c")
            )
            nc.gpsimd.dma_start(
                out=s1h_sb[:, 4:HC], in_=s1h_t[4:HC].rearrange("m k p c -> p m k c")
            )
            nc.gpsimd.dma_start(
                out=s1l_sb[:, 4:HC], in_=s1l_t[4:HC].rearrange("m k p c -> p m k c")
            )
            # remaining x blocks: hi on sync, lo on scalar
            for n in range(1, NSPLIT):
                nc.sync.dma_start(
                    out=xh_sb[:, n], in_=xh_t[n].rearrange("k p c -> p k c")
                )
                nc.scalar.dma_start(
                    out=xl_sb[:, n], in_=xl_t[n].rearrange("k p c -> p k c")
                )
            # small later-layer params (trivial bandwidth)
            nc.scalar.dma_start(out=w4_sb, in_=w4_t.rearrange("k p o -> p k o"))
            nc.scalar.dma_start(out=b2v, in_=b2_t[:].rearrange("(m p) -> p m", p=128))
            nc.scalar.dma_start(out=sc3v, in_=sc3_t[:].rearrange("(m p) -> p m", p=128))
            nc.scalar.dma_start(out=sh3v, in_=sh3_t[:].rearrange("(m p) -> p m", p=128))
            b4_ap = b4_t[:]
            nc.scalar.dma_start(
                out=b4bc,
                in_=bass.AP(tensor=b4_ap.tensor, offset=b4_ap.offset,
                            ap=[[0, 32]] + list(b4_ap.ap)),
            )

            # ---- fc1: h1*2^6 = (hi + tail + lo) contractions, sign -> act1
            x_gate = None
            for n in range(NSPLIT):
                for m in range(HC):
                    ps = psum.tile([128, NB], f32, tag="mm", name="ps")
                    for k in range(KH):
                        nc.tensor.matmul(
                            ps, s1h_sb[:, m, k], xh_sb[:, n, k],
                            start=(k == 0), stop=False,
                        )
                    nc.tensor.matmul(
                        ps, s1t_sb[:, m], xt_sb[:, n], start=False, stop=False
                    )
                    for j in range(KL // 2):
                        jsl = slice(2 * j, 2 * j + 2)
                        nc.tensor.matmul(
                            ps, s1l_sb[:, m, jsl], xl_sb[:, n, jsl],
                            start=False, stop=(j == KL // 2 - 1),
                            perf_mode=DR,
                        )
                    a = nc.scalar.activation(
                        act1[:, m, ts(n, NB)], ps, AF.Sign, bias=b1v[:, m:m + 1]
                    )
                    if n == 1 and m == 0:
                        x_gate = a

            # later-layer weights deferred until fc1 is past the DMA crunch
            for a0 in (0, 4):
                d = nc.gpsimd.dma_start(
                    out=s2_sb[:, a0:a0 + 4],
                    in_=s2_t[a0:a0 + 4].rearrange("m k p c -> p m k c"),
                )
                add_dep_helper(d.ins, x_gate.ins, reason="defer s2 after x load")
            for a0 in (0, 4):
                d = nc.gpsimd.dma_start(
                    out=s3_sb[:, a0:a0 + 4],
                    in_=s3_t[a0:a0 + 4].rearrange("m k p c -> p m k c"),
                )
                add_dep_helper(d.ins, x_gate.ins, reason="defer s3 after x load")

            # ---- fc2: binary x binary, fp8 DoubleRow, sign -> act2 ----
            for m in range(HC):
                pss = [psum.tile([128, NB], f32, tag="mm", name="ps") for _ in range(NSPLIT)]
                for kk in range(HC // 2):
                    ksl = slice(2 * kk, 2 * kk + 2)
                    for n in range(NSPLIT):
                        nc.tensor.matmul(
                            pss[n], s2_sb[:, m, ksl], act1[:, ksl, ts(n, NB)],
                            start=(kk == 0), stop=(kk == HC // 2 - 1),
                            perf_mode=DR,
                        )
                for n in range(NSPLIT):
                    nc.scalar.activation(
                        act2[:, m, ts(n, NB)], pss[n], AF.Sign, bias=b2v[:, m:m + 1]
                    )

            # ---- fc3: fp8 DoubleRow, bn affine + hardtanh -> act3 (DVE) ----
            for m in range(HC):
                pss = [psum.tile([128, NB], f32, tag="mm", name="ps") for _ in range(NSPLIT)]
                for kk in range(HC // 2):
                    ksl = slice(2 * kk, 2 * kk + 2)
                    for n in range(NSPLIT):
                        nc.tensor.matmul(
                            pss[n], s3_sb[:, m, ksl], act2[:, ksl, ts(n, NB)],
                            start=(kk == 0), stop=(kk == HC // 2 - 1),
                            perf_mode=DR,
                        )
                for n in range(NSPLIT):
                    t = tmp.tile([128, NB], f32, tag="t3")
                    nc.scalar.activation(
                        t, pss[n], AF.Identity,
                        bias=sh3v[:, m:m + 1], scale=sc3v[:, m:m + 1],
                    )
                    nc.vector.tensor_scalar(
                        out=act3[:, m, ts(n, NB)], in0=t,
                        scalar1=-1.0, scalar2=1.0,
                        op0=ALU.max, op1=ALU.min,
                    )

            # ---- fc4 + log_softmax per n-block ----
            # ltr[p, 32u+o] = logit class o of column 32u+p; column 32u+p
            # carries batch row 64p+u (host permutation), so partition p of
            # outf holds y rows 64p..64p+63 contiguously.
            yb = y_t[:]
            b4r = b4bc[:]
            for n in range(NSPLIT):
                ps4 = psum.tile([OUT_F, NB], f32, tag="mm", name="ps4")
                for k in range(HC):
                    nc.tensor.matmul(
                        ps4, w4_sb[:, k], act3[:, k, ts(n, NB)],
                        start=(k == 0), stop=(k == HC - 1),
                    )
                nc.scalar.copy(logits[0:OUT_F, ts(n, NB)], ps4)
                nc.vector.transpose(ltr[:, ts(n, NB)], logits[:, ts(n, NB)])
                base = ltr[:, ts(n, NB)]
                ltv = bass.AP(tensor=base.tensor, offset=base.offset,
                              ap=[base.ap[0], [32, NJ], [1, OUT_F]])
                nc.vector.tensor_tensor(
                    out=ltv, in0=ltv,
                    in1=bass.AP(tensor=b4r.tensor, offset=b4r.offset,
                                ap=[[b4r.ap[0][0], 32], [0, NJ], b4r.ap[1]]),
                    op=ALU.add,
                )
                esn = es[:, ts(n, NJ), :]
                nc.scalar.activation(esn, ltv, AF.Exp)
                lsen = lse[:, ts(n, NJ)]
                nc.vector.tensor_reduce(
                    out=lsen, in_=esn, axis=mybir.AxisListType.X, op=ALU.add
                )
                nc.scalar.activation(lsen, lsen, AF.Ln)
                outfn = outf[:, ts(n, NJ), :]
                lser = lsen
                nc.vector.tensor_tensor(
                    out=outfn, in0=ltv,
                    in1=bass.AP(tensor=lser.tensor, offset=lser.offset,
                                ap=[lser.ap[0], lser.ap[1], [0, OUT_F]]),
                    op=ALU.subtract,
                )
                # y rows 64p + [16n, 16n+16) per partition p: contiguous 640B
                nc.sync.dma_start(
                    out=bass.AP(tensor=yb.tensor, offset=yb.offset + NJ * OUT_F * n,
                                ap=[[64 * OUT_F, 32], [OUT_F, NJ], [1, OUT_F]]),
                    in_=outfn,
                )

    nc.finalize()
    return nc


def _host_prep(inputs):
    """Shard x, binarize/lay out weights, fold bn into sign biases."""
    import ml_dtypes

    f16 = np.float16
    f8 = ml_dtypes.float8_e4m3

    x = np.asarray(inputs["x"], np.float32)
    w1 = np.asarray(inputs["w1"], np.float32)
    w2 = np.asarray(inputs["w2"], np.float32)
    w3 = np.asarray(inputs["w3"], np.float32)
    w4 = np.asarray(inputs["w4"], np.float32)
    b1 = np.asarray(inputs["b1"], np.float32)
    b2 = np.asarray(inputs["b2"], np.float32)
    b3 = np.asarray(inputs["b3"], np.float32)
    b4 = np.asarray(inputs["b4"], np.float32)

    EPS = np.float64(1e-5)

    def gv(i):
        return (np.asarray(inputs[f"g{i}"], np.float32),
                np.asarray(inputs[f"be{i}"], np.float32),
                np.asarray(inputs[f"m{i}"], np.float32),
                np.asarray(inputs[f"v{i}"], np.float32))

    g1, be1, m1, v1 = gv(1)
    g2, be2, m2, v2 = gv(2)
    g3, be3, m3, v3 = gv(3)
    # sign(bn(h)) == sign(h + (b - m)) requires gamma > 0 and beta == 0
    assert np.all(g1 > 0) and np.all(be1 == 0), "unsupported bn1 params"
    assert np.all(g2 > 0) and np.all(be2 == 0), "unsupported bn2 params"

    bias1 = ((b1 - m1) * 64.0).astype(np.float32)   # fc1 runs at 2^6 scale
    bias2 = (b2 - m2).astype(np.float32)
    r3 = 1.0 / np.sqrt(v3.astype(np.float64) + EPS)
    sc3 = (r3 * g3).astype(np.float32)
    sh3 = ((b3 - m3).astype(np.float64) * r3 * g3 + be3).astype(np.float32)

    # fc1 weights: rows 0-767 split [m][k][p][c]; hi at sign*2^6 (f16),
    # lo at sign*2^-6 (fp8, min normal); tail rows 768-783 packed K=32
    # (sign*2^6 replicated at p0-15 / p16-31 for the hi/lo f16 tail data).
    s1f = np.sign(w1).T.astype(np.float32)              # [784, 1024]
    body = np.ascontiguousarray(
        s1f[:768].reshape(KH, 128, HC, 128).transpose(2, 0, 1, 3))
    s1h = (body * 64.0).astype(f16)                     # [8, 6, 128, 128]
    s1l = (body * (2.0 ** -6)).astype(f8)
    tail = s1f[768:IN_F].reshape(16, HC, 128) * 64.0    # [16, 8, 128]
    s1t = np.zeros((HC, 32, 128), np.float32)
    for mm in range(HC):
        s1t[mm, 0:16] = tail[:, mm]
        s1t[mm, 16:32] = tail[:, mm]
    s1t = s1t.astype(f16)

    def wlay(w, kc, dt):  # [out, in] -> [m, k, 128p(in), 128c(out)]
        st = np.sign(w).T.astype(np.float32)            # [in, out]
        mo = st.shape[1] // 128
        return np.ascontiguousarray(
            st.reshape(kc, 128, mo, 128).transpose(2, 0, 1, 3)
        ).astype(dt)

    s2t = wlay(w2, HC, f8)
    s3t = wlay(w3, HC, f8)
    w4t = np.ascontiguousarray(w4.T.astype(f16)).reshape(HC, 128, OUT_F)

    shared = dict(s1h=s1h, s1t=s1t, s1l=s1l, s2t=s2t, s3t=s3t, w4t=w4t,
                  bias1=bias1, bias2=bias2, sc3=sc3, sh3=sh3, b4=b4)

    # column permutation: kernel column c carries batch row 64*(c%32)+c//32,
    # so the transposed output lands contiguously per partition.
    cc = np.arange(BS)
    perm = 64 * (cc % 32) + cc // 32

    in_maps = []
    for c in range(N_CORES):
        xs = x[c * BS:(c + 1) * BS][perm]               # [2048, 784] permuted
        xcols = np.ascontiguousarray(xs.T)              # [784, 2048] fp32
        xhi = xcols.astype(f16)
        lo32 = xcols - xhi.astype(np.float32)           # exact residual
        xh = np.ascontiguousarray(
            xhi[:768].reshape(KH, 128, NSPLIT, NB).transpose(2, 0, 1, 3))
        xl = np.ascontiguousarray(
            (lo32[:768] * 4096.0).reshape(KL, 128, NSPLIT, NB)
            .transpose(2, 0, 1, 3)).astype(f8)
        xt = np.empty((32, BS), f16)
        xt[0:16] = xhi[768:IN_F]
        xt[16:32] = lo32[768:IN_F].astype(f16)
        xt = np.ascontiguousarray(xt.reshape(32, NSPLIT, NB).transpose(1, 0, 2))
        m = dict(shared)
        m["xh"] = xh
        m["xl"] = xl
        m["xt"] = xt
        in_maps.append(m)
    return in_maps


def kernel(**inputs):
    global LAST_RESULT
    from concourse.bass_utils import run_bass_kernel_spmd

    if "nc" not in _PLAN:
        _PLAN["nc"] = _build_nc()
    nc = _PLAN["nc"]

    in_maps = _host_prep(inputs)
    br = run_bass_kernel_spmd(
        nc, in_maps, list(range(N_CORES)),
        tmpdir=os.environ.get("KERNEL_TMPDIR") or None,
    )
    LAST_RESULT = br
    out = np.concatenate([br.results[c]["y"] for c in range(N_CORES)], axis=0)
    return out.astype(np.float32)


# revision 4
# speedup vs baseline: 1.1018x; 1.0165x over previous
"""Binarized MLP (784 -> 1024 -> 1024 -> 1024 -> 10) on 8 TRN2 NeuronCores.

Data-parallel over the batch (16384 rows -> 2048 per core), weights replicated.

Math notes:
  * Layers 1-2 outputs are only consumed through binarize(hardtanh(bn(h))).
    With g > 0, be == 0 that is exactly sign(h + (b - m)): one ScalarE Sign
    activation with a per-partition bias.
  * fc2/fc3 multiply two +-1 operands -> exact in fp8(e4m3) with fp32 PSUM
    accumulation; DoubleRow contracts 256 rows per 512-cycle pass.
  * fc1 keeps x near-full precision in 10 PE passes (vs 13 for exact hi/lo):
    7 f16 hi chunks (rows 768-783 ride chunk 6 as a packed hi/lo pair at
    p0-15/p16-31, zero elsewhere) plus an fp8 residual lo = x - f16(x)
    scaled by 2^12 contracted in 3 fp8-DR passes with weights sign*2^-6;
    the whole layer runs at a 2^6 global scale (hi weights sign*64, bias*64)
    so every fp8 value stays in the normal range. Sign() is scale-invariant.
    Host-simulated rel err: 9.4e-3 (22 act1 sign flips of 16.7M).
  * fc1 runs n-outer with PHASE-UNIFORM matmul modes: per 512-col block all
    56 f16 matmuls accumulate into 8 pinned PSUM banks, then all 24 fp8-DR
    matmuls close them. LDWEIGHTS only overlaps execution when the PE config
    (dtype/tile/perf-mode) matches, so interleaving modes costs ~260ns/tile.
  * Batch columns are host-permuted (col c carries batch row 64*(c%32)+c//32)
    so after the DVE 32x32 output transpose each partition holds contiguous
    y rows: the output DMA is 32 x 640B runs instead of 2048 x 40B scatter.
  * fc4 + log_softmax run per 512-col block overlapping later fc4 matmuls.
    The PSUM->SBUF logits copy rides the DVE so ScalarE's activation table
    switches exactly once (Sign/Identity set -> Exp/Ln set).
  * Ramp-critical weights (s1 hi m0-3, lo m0) ride the two HWDGE rings with
    x; the gpsimd SWDGE ring (which starts ~4us late) only carries s2/s3
    and small vectors.
"""

import os
import numpy as np

N_CORES = 8
B_FULL = 16384
BS = B_FULL // N_CORES  # 2048 rows per core
IN_F = 784
H = 1024
HC = 8                  # hidden chunks of 128
OUT_F = 10
NSPLIT = 4              # batch column blocks of 512
NB = BS // NSPLIT       # 512
KH = 7                  # f16 hi chunks (6 full + packed tail chunk)
KL = 6                  # fp8 lo chunks = 3 DoubleRow pairs (rows 0-767)
NJ = NB // 32           # 16 32-col blocks per n-block (output transpose)

LAST_RESULT = None      # BassKernelResults of the most recent run (for test.py)

_PLAN = {}


def _build_nc():
    import concourse.bass as bass
    import concourse.mybir as mybir
    import concourse.tile as tile
    from concourse.tile import add_dep_helper
    from concourse import bacc
    from concourse.bass import ts

    f32 = mybir.dt.float32
    f16 = mybir.dt.float16
    f8 = mybir.dt.float8e4
    AF = mybir.ActivationFunctionType
    ALU = mybir.AluOpType
    DR = mybir.MatmulPerfMode.DoubleRow

    nc = bacc.Bacc(None)

    xh_t = nc.dram_tensor("xh", [NSPLIT, KH, 128, NB], f16, kind="ExternalInput")
    xl_t = nc.dram_tensor("xl", [NSPLIT, KL, 128, NB], f8, kind="ExternalInput")
    s1h_t = nc.dram_tensor("s1h", [HC, KH, 128, 128], f16, kind="ExternalInput")
    s1l_t = nc.dram_tensor("s1l", [HC, KL, 128, 128], f8, kind="ExternalInput")
    s2_t = nc.dram_tensor("s2t", [HC, HC, 128, 128], f8, kind="ExternalInput")
    s3_t = nc.dram_tensor("s3t", [HC, HC, 128, 128], f8, kind="ExternalInput")
    w4_t = nc.dram_tensor("w4t", [HC, 128, OUT_F], f16, kind="ExternalInput")
    b1_t = nc.dram_tensor("bias1", [H], f32, kind="ExternalInput")
    b2_t = nc.dram_tensor("bias2", [H], f32, kind="ExternalInput")
    sc3_t = nc.dram_tensor("sc3", [H], f32, kind="ExternalInput")
    sh3_t = nc.dram_tensor("sh3", [H], f32, kind="ExternalInput")
    b4_t = nc.dram_tensor("b4", [OUT_F], f32, kind="ExternalInput")
    y_t = nc.dram_tensor("y", [BS, OUT_F], f32, kind="ExternalOutput")

    with tile.TileContext(nc) as tc:
        with (
            tc.tile_pool(name="consts", bufs=1) as consts,
            tc.tile_pool(name="tmp", bufs=4) as tmp,
            tc.tile_pool(name="psum", bufs=8, space="PSUM") as psum,
        ):
            xh_sb = consts.tile([128, NSPLIT, KH, NB], f16, tag="xh")
            xl_sb = consts.tile([128, NSPLIT, KL, NB], f8, tag="xl")
            s1h_sb = consts.tile([128, HC, KH, 128], f16, tag="s1h")
            s1l_sb = consts.tile([128, HC, KL, 128], f8, tag="s1l")
            s2_sb = consts.tile([128, HC, HC, 128], f8, tag="s2")
            s3_sb = consts.tile([128, HC, HC, 128], f8, tag="s3")
            w4_sb = consts.tile([128, HC, OUT_F], f16, tag="w4")
            b1v = consts.tile([128, HC], f32, tag="b1v")
            b2v = consts.tile([128, HC], f32, tag="b2v")
            sc3v = consts.tile([128, HC], f32, tag="sc3v")
            sh3v = consts.tile([128, HC], f32, tag="sh3v")
            b4bc = consts.tile([32, OUT_F], f32, tag="b4bc")
            act1 = consts.tile([128, HC, BS], f8, tag="act1")
            act2 = consts.tile([128, HC, BS], f8, tag="act2")
            act3 = consts.tile([128, HC, BS], f16, tag="act3")
            logits = consts.tile([32, BS], f32, tag="logits")
            ltr = consts.tile([32, BS], f32, tag="ltr")
            es = consts.tile([32, NSPLIT * NJ, OUT_F], f32, tag="es")
            lse = consts.tile([32, NSPLIT * NJ], f32, tag="lse")
            outf = consts.tile([32, NSPLIT * NJ, OUT_F], f32, tag="outf")

            # ---- ramp DMAs, priority order. The two HWDGE rings (sync/
            # scalar) carry everything fc1-n0 needs; gpsimd (SWDGE, ~4us
            # startup lag) carries only small vectors and gated s2/s3.
            nc.sync.dma_start(
                out=s1h_sb[:, 0], in_=s1h_t[0].rearrange("k p c -> p k c")
            )
            nc.scalar.dma_start(
                out=xh_sb[:, 0, 4:KH], in_=xh_t[0, 4:KH].rearrange("k p c -> p k c")
            )
            nc.sync.dma_start(
                out=xh_sb[:, 0, 0:4], in_=xh_t[0, 0:4].rearrange("k p c -> p k c")
            )
            nc.gpsimd.dma_start(out=b1v, in_=b1_t[:].rearrange("(m p) -> p m", p=128))
            nc.scalar.dma_start(
                out=s1l_sb[:, 0:4], in_=s1l_t[0:4].rearrange("m k p c -> p m k c")
            )
            nc.sync.dma_start(
                out=s1h_sb[:, 1:4], in_=s1h_t[1:4].rearrange("m k p c -> p m k c")
            )
            nc.scalar.dma_start(
                out=s1h_sb[:, 4:HC], in_=s1h_t[4:HC].rearrange("m k p c -> p m k c")
            )
            nc.sync.dma_start(
                out=xl_sb[:, 0], in_=xl_t[0].rearrange("k p c -> p k c")
            )
            nc.scalar.dma_start(
                out=s1l_sb[:, 4:HC], in_=s1l_t[4:HC].rearrange("m k p c -> p m k c")
            )
            nc.vector.memset(logits, 0.0)
            # remaining x blocks split across the rings
            nc.sync.dma_start(out=xh_sb[:, 1], in_=xh_t[1].rearrange("k p c -> p k c"))
            nc.scalar.dma_start(out=xl_sb[:, 1], in_=xl_t[1].rearrange("k p c -> p k c"))
            nc.scalar.dma_start(out=xh_sb[:, 2], in_=xh_t[2].rearrange("k p c -> p k c"))
            nc.sync.dma_start(out=xl_sb[:, 2], in_=xl_t[2].rearrange("k p c -> p k c"))
            nc.sync.dma_start(out=xh_sb[:, 3], in_=xh_t[3].rearrange("k p c -> p k c"))
            nc.scalar.dma_start(out=xl_sb[:, 3], in_=xl_t[3].rearrange("k p c -> p k c"))
            # small later-layer params (trivial bandwidth)
            nc.scalar.dma_start(out=w4_sb, in_=w4_t.rearrange("k p o -> p k o"))
            nc.sync.dma_start(out=b2v, in_=b2_t[:].rearrange("(m p) -> p m", p=128))
            nc.sync.dma_start(out=sc3v, in_=sc3_t[:].rearrange("(m p) -> p m", p=128))
            nc.sync.dma_start(out=sh3v, in_=sh3_t[:].rearrange("(m p) -> p m", p=128))
            b4_ap = b4_t[:]
            nc.sync.dma_start(
                out=b4bc,
                in_=bass.AP(tensor=b4_ap.tensor, offset=b4_ap.offset,
                            ap=[[0, 32]] + list(b4_ap.ap)),
            )

            # ---- fc1: phase-uniform per n-block; 8 PSUM banks pinned.
            # h1*2^6 = hi-chunk contractions (f16) then lo residual (fp8 DR),
            # Sign -> act1.
            x_gate = None
            for n in range(NSPLIT):
                pss = [psum.tile([128, NB], f32, tag="mm", name="ps") for _ in range(HC)]
                for m in range(HC):
                    for k in range(KH):
                        nc.tensor.matmul(
                            pss[m], s1h_sb[:, m, k], xh_sb[:, n, k],
                            start=(k == 0), stop=False,
                        )
                for m in range(HC):
                    for j in range(KL // 2):
                        jsl = slice(2 * j, 2 * j + 2)
                        nc.tensor.matmul(
                            pss[m], s1l_sb[:, m, jsl], xl_sb[:, n, jsl],
                            start=False, stop=(j == KL // 2 - 1),
                            perf_mode=DR,
                        )
                    a = nc.scalar.activation(
                        act1[:, m, ts(n, NB)], pss[m], AF.Sign, bias=b1v[:, m:m + 1]
                    )
                    if n == 1 and m == 0:
                        x_gate = a

            # later-layer weights deferred until fc1 is past the DMA crunch
            for a0 in (0, 4):
                d = nc.gpsimd.dma_start(
                    out=s2_sb[:, a0:a0 + 4],
                    in_=s2_t[a0:a0 + 4].rearrange("m k p c -> p m k c"),
                )
                add_dep_helper(d.ins, x_gate.ins, reason="defer s2 after x load")
            for a0 in (0, 4):
                d = nc.gpsimd.dma_start(
                    out=s3_sb[:, a0:a0 + 4],
                    in_=s3_t[a0:a0 + 4].rearrange("m k p c -> p m k c"),
                )
                add_dep_helper(d.ins, x_gate.ins, reason="defer s3 after x load")

            # ---- fc2: binary x binary, fp8 DoubleRow, sign -> act2 ----
            for m in range(HC):
                pss = [psum.tile([128, NB], f32, tag="mm", name="ps") for _ in range(NSPLIT)]
                for kk in range(HC // 2):
                    ksl = slice(2 * kk, 2 * kk + 2)
                    for n in range(NSPLIT):
                        nc.tensor.matmul(
                            pss[n], s2_sb[:, m, ksl], act1[:, ksl, ts(n, NB)],
                            start=(kk == 0), stop=(kk == HC // 2 - 1),
                            perf_mode=DR,
                        )
                for n in range(NSPLIT):
                    nc.scalar.activation(
                        act2[:, m, ts(n, NB)], pss[n], AF.Sign, bias=b2v[:, m:m + 1]
                    )

            # ---- fc3: fp8 DoubleRow, bn affine + hardtanh -> act3 (DVE) ----
            for m in range(HC):
                pss = [psum.tile([128, NB], f32, tag="mm", name="ps") for _ in range(NSPLIT)]
                for kk in range(HC // 2):
                    ksl = slice(2 * kk, 2 * kk + 2)
                    for n in range(NSPLIT):
                        nc.tensor.matmul(
                            pss[n], s3_sb[:, m, ksl], act2[:, ksl, ts(n, NB)],
                            start=(kk == 0), stop=(kk == HC // 2 - 1),
                            perf_mode=DR,
                        )
                for n in range(NSPLIT):
                    t = tmp.tile([128, NB], f32, tag="t3")
                    nc.scalar.activation(
                        t, pss[n], AF.Identity,
                        bias=sh3v[:, m:m + 1], scale=sc3v[:, m:m + 1],
                    )
                    nc.vector.tensor_scalar(
                        out=act3[:, m, ts(n, NB)], in0=t,
                        scalar1=-1.0, scalar2=1.0,
                        op0=ALU.max, op1=ALU.min,
                    )

            # ---- fc4 + log_softmax per n-block ----
            # ltr[p, 32u+o] = logit class o of column 32u+p; column 32u+p
            # carries batch row 64p+u (host permutation), so partition p of
            # outf holds y rows 64p..64p+63 contiguously. ScalarE only runs
            # Exp/Ln here (single activation-table switch); the PSUM->SBUF
            # copy rides the DVE.
            yb = y_t[:]
            b4r = b4bc[:]
            for n in range(NSPLIT):
                ps4 = psum.tile([OUT_F, NB], f32, tag="mm", name="ps4")
                for k in range(HC):
                    nc.tensor.matmul(
                        ps4, w4_sb[:, k], act3[:, k, ts(n, NB)],
                        start=(k == 0), stop=(k == HC - 1),
                    )
                nc.vector.tensor_scalar_add(
                    out=logits[0:OUT_F, ts(n, NB)], in0=ps4, scalar1=0.0
                )
                nc.vector.transpose(ltr[:, ts(n, NB)], logits[:, ts(n, NB)])
                base = ltr[:, ts(n, NB)]
                ltv = bass.AP(tensor=base.tensor, offset=base.offset,
                              ap=[base.ap[0], [32, NJ], [1, OUT_F]])
                nc.vector.tensor_tensor(
                    out=ltv, in0=ltv,
                    in1=bass.AP(tensor=b4r.tensor, offset=b4r.offset,
                                ap=[[b4r.ap[0][0], 32], [0, NJ], b4r.ap[1]]),
                    op=ALU.add,
                )
                esn = es[:, ts(n, NJ), :]
                nc.scalar.activation(esn, ltv, AF.Exp)
                lsen = lse[:, ts(n, NJ)]
                nc.vector.tensor_reduce(
                    out=lsen, in_=esn, axis=mybir.AxisListType.X, op=ALU.add
                )
                nc.scalar.activation(lsen, lsen, AF.Ln)
                outfn = outf[:, ts(n, NJ), :]
                lser = lsen
                nc.vector.tensor_tensor(
                    out=outfn, in0=ltv,
                    in1=bass.AP(tensor=lser.tensor, offset=lser.offset,
                                ap=[lser.ap[0], lser.ap[1], [0, OUT_F]]),
                    op=ALU.subtract,
                )
                # y rows 64p + [16n, 16n+16) per partition p: contiguous 640B
                nc.sync.dma_start(
                    out=bass.AP(tensor=yb.tensor, offset=yb.offset + NJ * OUT_F * n,
                                ap=[[64 * OUT_F, 32], [OUT_F, NJ], [1, OUT_F]]),
                    in_=outfn,
                )

    nc.finalize()
    return nc


def _host_prep(inputs):
    """Shard x, binarize/lay out weights, fold bn into sign biases."""
    import ml_dtypes

    f16 = np.float16
    f8 = ml_dtypes.float8_e4m3

    x = np.asarray(inputs["x"], np.float32)
    w1 = np.asarray(inputs["w1"], np.float32)
    w2 = np.asarray(inputs["w2"], np.float32)
    w3 = np.asarray(inputs["w3"], np.float32)
    w4 = np.asarray(inputs["w4"], np.float32)
    b1 = np.asarray(inputs["b1"], np.float32)
    b2 = np.asarray(inputs["b2"], np.float32)
    b3 = np.asarray(inputs["b3"], np.float32)
    b4 = np.asarray(inputs["b4"], np.float32)

    EPS = np.float64(1e-5)

    def gv(i):
        return (np.asarray(inputs[f"g{i}"], np.float32),
                np.asarray(inputs[f"be{i}"], np.float32),
                np.asarray(inputs[f"m{i}"], np.float32),
                np.asarray(inputs[f"v{i}"], np.float32))

    g1, be1, m1, v1 = gv(1)
    g2, be2, m2, v2 = gv(2)
    g3, be3, m3, v3 = gv(3)
    # sign(bn(h)) == sign(h + (b - m)) requires gamma > 0 and beta == 0
    assert np.all(g1 > 0) and np.all(be1 == 0), "unsupported bn1 params"
    assert np.all(g2 > 0) and np.all(be2 == 0), "unsupported bn2 params"

    bias1 = ((b1 - m1) * 64.0).astype(np.float32)   # fc1 runs at 2^6 scale
    bias2 = (b2 - m2).astype(np.float32)
    r3 = 1.0 / np.sqrt(v3.astype(np.float64) + EPS)
    sc3 = (r3 * g3).astype(np.float32)
    sh3 = ((b3 - m3).astype(np.float64) * r3 * g3 + be3).astype(np.float32)

    # fc1 weights: rows 0-767 split [m][k][p][c]; hi at sign*2^6 (f16),
    # lo at sign*2^-6 (fp8, min normal). Hi chunk 6 is the packed tail:
    # rows 768-783 replicated at p0-15 and p16-31 (hi/lo), zero elsewhere.
    s1f = np.sign(w1).T.astype(np.float32)              # [784, 1024]
    body = s1f[:768].reshape(6, 128, HC, 128).transpose(2, 0, 1, 3)
    s1h = np.zeros((HC, KH, 128, 128), np.float32)
    s1h[:, :6] = body * 64.0
    tail = s1f[768:IN_F].reshape(16, HC, 128) * 64.0    # [16, 8, 128]
    for mm in range(HC):
        s1h[mm, 6, 0:16] = tail[:, mm]
        s1h[mm, 6, 16:32] = tail[:, mm]
    s1h = s1h.astype(f16)
    s1l = np.ascontiguousarray(body * (2.0 ** -6)).astype(f8)

    def wlay(w, kc, dt):  # [out, in] -> [m, k, 128p(in), 128c(out)]
        st = np.sign(w).T.astype(np.float32)            # [in, out]
        mo = st.shape[1] // 128
        return np.ascontiguousarray(
            st.reshape(kc, 128, mo, 128).transpose(2, 0, 1, 3)
        ).astype(dt)

    s2t = wlay(w2, HC, f8)
    s3t = wlay(w3, HC, f8)
    w4t = np.ascontiguousarray(w4.T.astype(f16)).reshape(HC, 128, OUT_F)

    shared = dict(s1h=s1h, s1l=s1l, s2t=s2t, s3t=s3t, w4t=w4t,
                  bias1=bias1, bias2=bias2, sc3=sc3, sh3=sh3, b4=b4)

    # column permutation: kernel column c carries batch row 64*(c%32)+c//32,
    # so the transposed output lands contiguously per partition.
    cc = np.arange(BS)
    perm = 64 * (cc % 32) + cc // 32

    in_maps = []
    for c in range(N_CORES):
        xs = x[c * BS:(c + 1) * BS][perm]               # [2048, 784] permuted
        xcols = np.ascontiguousarray(xs.T)              # [784, 2048] fp32
        xhi = xcols.astype(f16)
        lo32 = xcols - xhi.astype(np.float32)           # exact residual
        xh = np.zeros((KH, 128, BS), f16)
        xh[:6] = xhi[:768].reshape(6, 128, BS)
        xh[6, 0:16] = xhi[768:IN_F]
        xh[6, 16:32] = lo32[768:IN_F].astype(f16)
        xh = np.ascontiguousarray(
            xh.reshape(KH, 128, NSPLIT, NB).transpose(2, 0, 1, 3))
        xl = np.ascontiguousarray(
            (lo32[:768] * 4096.0).reshape(KL, 128, NSPLIT, NB)
            .transpose(2, 0, 1, 3)).astype(f8)
        m = dict(shared)
        m["xh"] = xh
        m["xl"] = xl
        in_maps.append(m)
    return in_maps


def kernel(**inputs):
    global LAST_RESULT
    from concourse.bass_utils import run_bass_kernel_spmd

    if "nc" not in _PLAN:
        _PLAN["nc"] = _build_nc()
    nc = _PLAN["nc"]

    in_maps = _host_prep(inputs)
    br = run_bass_kernel_spmd(
        nc, in_maps, list(range(N_CORES)),
        tmpdir=os.environ.get("KERNEL_TMPDIR") or None,
    )
    LAST_RESULT = br
    out = np.concatenate([br.results[c]["y"] for c in range(N_CORES)], axis=0)
    return out.astype(np.float32)


# revision 6
# speedup vs baseline: 1.1793x; 1.0703x over previous
"""Binarized MLP (784 -> 1024 -> 1024 -> 1024 -> 10) on 8 TRN2 NeuronCores.

Data-parallel over the batch (16384 rows -> 2048 per core), weights replicated.

Math notes:
  * Layers 1-2 outputs are only consumed through binarize(hardtanh(bn(h))).
    With g > 0, be == 0 that is exactly sign(h + (b - m)): one ScalarE Sign
    activation with a per-partition bias.
  * fc2/fc3 multiply two +-1 operands -> exact in fp8(e4m3) with fp32 PSUM
    accumulation; DoubleRow contracts 256 rows per 512-cycle pass.
  * fc1 keeps x near-full precision in 10 PE passes (vs 13 for exact hi/lo):
    7 f16 hi chunks (rows 768-783 ride chunk 6 as a packed hi/lo pair at
    p0-15/p16-31, zero elsewhere) plus an fp8 residual lo = x - f16(x)
    scaled by 2^12 contracted in 3 fp8-DR passes with weights sign*2^-6;
    the whole layer runs at a 2^6 global scale (hi weights sign*64, bias*64)
    so every fp8 value stays in the normal range. Sign() is scale-invariant.
    Host-simulated rel err: 9.4e-3 (22 act1 sign flips of 16.7M).
  * fc1 runs n-outer with PHASE-UNIFORM matmul modes: per 512-col block all
    56 f16 matmuls accumulate into 8 pinned PSUM banks, then all 24 fp8-DR
    matmuls close them. LDWEIGHTS only overlaps execution when the PE config
    (dtype/tile/perf-mode) matches, so interleaving modes costs ~260ns/tile.
  * ALL dram tensors are stored partition-major (host relayout) so every DMA
    is 128 descriptor rows of contiguous KBs: a `m k p c -> p m k c` gather
    costs ~2.4ns per 128-256B descriptor row on the issuing engine (a 1MB
    weight DMA = 7-9us of issue time, serializing its ring).
  * Batch columns are host-permuted (col c carries batch row 64*(c%32)+c//32)
    so after the DVE 32x32 output transpose each partition holds contiguous
    y rows: the output DMA is 32 x 640B runs instead of 2048 x 40B scatter.
  * fc4 + log_softmax: per n-block copy (DVE) / transpose / +b4 / Exp /
    reduce overlap later fc4 matmuls; the four Ln (a DIFFERENT activation
    table set than Exp!) run batched at the end so ScalarE swaps tables
    exactly twice in the tail instead of 8 times at 1.28us each.
"""

import os
import numpy as np

N_CORES = 8
B_FULL = 16384
BS = B_FULL // N_CORES  # 2048 rows per core
IN_F = 784
H = 1024
HC = 8                  # hidden chunks of 128
OUT_F = 10
NSPLIT = 4              # batch column blocks of 512
NB = BS // NSPLIT       # 512
KH = 7                  # f16 hi chunks (6 full + packed tail chunk)
KL = 6                  # fp8 lo chunks = 3 DoubleRow pairs (rows 0-767)
NJ = NB // 32           # 16 32-col blocks per n-block (output transpose)

LAST_RESULT = None      # BassKernelResults of the most recent run (for test.py)

_PLAN = {}


def _build_nc():
    import concourse.bass as bass
    import concourse.mybir as mybir
    import concourse.tile as tile
    from concourse.tile import add_dep_helper
    from concourse import bacc
    from concourse.bass import ts

    f32 = mybir.dt.float32
    f16 = mybir.dt.float16
    f8 = mybir.dt.float8e4
    AF = mybir.ActivationFunctionType
    ALU = mybir.AluOpType
    DR = mybir.MatmulPerfMode.DoubleRow

    nc = bacc.Bacc(None)

    # all layouts partition-major: leading dim 128 (or 32) is the SBUF
    # partition, everything after is contiguous per partition.
    xh_t = nc.dram_tensor("xh", [128, NSPLIT, KH, NB], f16, kind="ExternalInput")
    xl_t = nc.dram_tensor("xl", [128, NSPLIT, KL, NB], f8, kind="ExternalInput")
    s1h_t = nc.dram_tensor("s1h", [128, HC, KH, 128], f16, kind="ExternalInput")
    s1l_t = nc.dram_tensor("s1l", [128, HC, KL, 128], f8, kind="ExternalInput")
    s2_t = nc.dram_tensor("s2t", [128, HC, HC, 128], f8, kind="ExternalInput")
    s3_t = nc.dram_tensor("s3t", [128, HC, HC, 128], f8, kind="ExternalInput")
    w4_t = nc.dram_tensor("w4t", [128, HC, OUT_F], f16, kind="ExternalInput")
    b1_t = nc.dram_tensor("bias1", [128, HC], f32, kind="ExternalInput")
    b2_t = nc.dram_tensor("bias2", [128, HC], f32, kind="ExternalInput")
    sc3_t = nc.dram_tensor("sc3", [128, HC], f32, kind="ExternalInput")
    sh3_t = nc.dram_tensor("sh3", [128, HC], f32, kind="ExternalInput")
    b4_t = nc.dram_tensor("b4", [OUT_F], f32, kind="ExternalInput")
    y_t = nc.dram_tensor("y", [BS, OUT_F], f32, kind="ExternalOutput")

    with tile.TileContext(nc) as tc:
        with (
            tc.tile_pool(name="consts", bufs=1) as consts,
            tc.tile_pool(name="tmp", bufs=4) as tmp,
            tc.tile_pool(name="psum", bufs=8, space="PSUM") as psum,
        ):
            xh_sb = consts.tile([128, NSPLIT, KH, NB], f16, tag="xh")
            xl_sb = consts.tile([128, NSPLIT, KL, NB], f8, tag="xl")
            s1h_sb = consts.tile([128, HC, KH, 128], f16, tag="s1h")
            s1l_sb = consts.tile([128, HC, KL, 128], f8, tag="s1l")
            s2_sb = consts.tile([128, HC, HC, 128], f8, tag="s2")
            s3_sb = consts.tile([128, HC, HC, 128], f8, tag="s3")
            w4_sb = consts.tile([128, HC, OUT_F], f16, tag="w4")
            b1v = consts.tile([128, HC], f32, tag="b1v")
            b2v = consts.tile([128, HC], f32, tag="b2v")
            sc3v = consts.tile([128, HC], f32, tag="sc3v")
            sh3v = consts.tile([128, HC], f32, tag="sh3v")
            b4bc = consts.tile([32, OUT_F], f32, tag="b4bc")
            act1 = consts.tile([128, HC, BS], f8, tag="act1")
            act2 = consts.tile([128, HC, BS], f8, tag="act2")
            act3 = consts.tile([128, HC, BS], f16, tag="act3")
            logits = consts.tile([32, BS], f32, tag="logits")
            ltr = consts.tile([32, BS], f32, tag="ltr")
            es = consts.tile([32, NSPLIT * NJ, OUT_F], f32, tag="es")
            lse = consts.tile([32, NSPLIT * NJ], f32, tag="lse")
            outf = consts.tile([32, NSPLIT * NJ, OUT_F], f32, tag="outf")

            # ---- ramp DMAs, priority order across the two HWDGE rings.
            # n0's x rides per-chunk so the PE can chase arrivals; weights
            # m0 first, then m1-3, m4-7 (needed ~1.5us per m-tile).
            nc.sync.dma_start(out=s1h_sb[:, 0:1], in_=s1h_t[:, 0:1])
            nc.scalar.dma_start(out=xh_sb[:, 0, 4:KH], in_=xh_t[:, 0, 4:KH])
            nc.sync.dma_start(out=xh_sb[:, 0, 0:2], in_=xh_t[:, 0, 0:2])
            nc.scalar.dma_start(out=s1l_sb[:, 0:4], in_=s1l_t[:, 0:4])
            nc.sync.dma_start(out=s1h_sb[:, 1:2], in_=s1h_t[:, 1:2])
            nc.sync.dma_start(out=xh_sb[:, 0, 2:4], in_=xh_t[:, 0, 2:4])
            nc.sync.dma_start(out=s1h_sb[:, 2:4], in_=s1h_t[:, 2:4])
            nc.scalar.dma_start(out=s1h_sb[:, 4:HC], in_=s1h_t[:, 4:HC])
            nc.sync.dma_start(out=xl_sb[:, 0], in_=xl_t[:, 0])
            nc.scalar.dma_start(out=b1v, in_=b1_t[:])
            nc.scalar.dma_start(out=s1l_sb[:, 4:HC], in_=s1l_t[:, 4:HC])
            nc.vector.memset(logits, 0.0)
            # remaining x blocks split across the rings
            nc.sync.dma_start(out=xh_sb[:, 1], in_=xh_t[:, 1])
            nc.scalar.dma_start(out=xl_sb[:, 1], in_=xl_t[:, 1])
            nc.scalar.dma_start(out=xh_sb[:, 2], in_=xh_t[:, 2])
            nc.sync.dma_start(out=xl_sb[:, 2], in_=xl_t[:, 2])
            nc.sync.dma_start(out=xh_sb[:, 3], in_=xh_t[:, 3])
            nc.scalar.dma_start(out=xl_sb[:, 3], in_=xl_t[:, 3])
            # small later-layer params (trivial bandwidth)
            nc.scalar.dma_start(out=w4_sb, in_=w4_t[:])
            nc.sync.dma_start(out=b2v, in_=b2_t[:])
            nc.sync.dma_start(out=sc3v, in_=sc3_t[:])
            nc.sync.dma_start(out=sh3v, in_=sh3_t[:])
            b4_ap = b4_t[:]
            nc.sync.dma_start(
                out=b4bc,
                in_=bass.AP(tensor=b4_ap.tensor, offset=b4_ap.offset,
                            ap=[[0, 32]] + list(b4_ap.ap)),
            )

            # ---- fc1: phase-uniform per n-block; 8 PSUM banks pinned.
            # h1*2^6 = hi-chunk contractions (f16) then lo residual (fp8 DR),
            # Sign -> act1.
            x_gate = None
            for n in range(NSPLIT):
                pss = [psum.tile([128, NB], f32, tag="mm", name="ps") for _ in range(HC)]
                for m in range(HC):
                    for k in range(KH):
                        nc.tensor.matmul(
                            pss[m], s1h_sb[:, m, k], xh_sb[:, n, k],
                            start=(k == 0), stop=False,
                        )
                for m in range(HC):
                    for j in range(KL // 2):
                        jsl = slice(2 * j, 2 * j + 2)
                        nc.tensor.matmul(
                            pss[m], s1l_sb[:, m, jsl], xl_sb[:, n, jsl],
                            start=False, stop=(j == KL // 2 - 1),
                            perf_mode=DR,
                        )
                    a = nc.scalar.activation(
                        act1[:, m, ts(n, NB)], pss[m], AF.Sign, bias=b1v[:, m:m + 1]
                    )
                    if n == 1 and m == 0:
                        x_gate = a

            # later-layer weights deferred until fc1 is past the DMA crunch
            for a0 in (0, 4):
                d = nc.gpsimd.dma_start(out=s2_sb[:, a0:a0 + 4], in_=s2_t[:, a0:a0 + 4])
                add_dep_helper(d.ins, x_gate.ins, reason="defer s2 after x load")
            for a0 in (0, 4):
                d = nc.gpsimd.dma_start(out=s3_sb[:, a0:a0 + 4], in_=s3_t[:, a0:a0 + 4])
                add_dep_helper(d.ins, x_gate.ins, reason="defer s3 after x load")

            # ---- fc2: binary x binary, fp8 DoubleRow, sign -> act2 ----
            for m in range(HC):
                pss = [psum.tile([128, NB], f32, tag="mm", name="ps") for _ in range(NSPLIT)]
                for kk in range(HC // 2):
                    ksl = slice(2 * kk, 2 * kk + 2)
                    for n in range(NSPLIT):
                        nc.tensor.matmul(
                            pss[n], s2_sb[:, m, ksl], act1[:, ksl, ts(n, NB)],
                            start=(kk == 0), stop=(kk == HC // 2 - 1),
                            perf_mode=DR,
                        )
                for n in range(NSPLIT):
                    nc.scalar.activation(
                        act2[:, m, ts(n, NB)], pss[n], AF.Sign, bias=b2v[:, m:m + 1]
                    )

            # ---- fc3: fp8 DoubleRow, bn affine + hardtanh -> act3 (DVE) ----
            for m in range(HC):
                pss = [psum.tile([128, NB], f32, tag="mm", name="ps") for _ in range(NSPLIT)]
                for kk in range(HC // 2):
                    ksl = slice(2 * kk, 2 * kk + 2)
                    for n in range(NSPLIT):
                        nc.tensor.matmul(
                            pss[n], s3_sb[:, m, ksl], act2[:, ksl, ts(n, NB)],
                            start=(kk == 0), stop=(kk == HC // 2 - 1),
                            perf_mode=DR,
                        )
                for n in range(NSPLIT):
                    t = tmp.tile([128, NB], f32, tag="t3")
                    nc.scalar.activation(
                        t, pss[n], AF.Identity,
                        bias=sh3v[:, m:m + 1], scale=sc3v[:, m:m + 1],
                    )
                    nc.vector.tensor_scalar(
                        out=act3[:, m, ts(n, NB)], in0=t,
                        scalar1=-1.0, scalar2=1.0,
                        op0=ALU.max, op1=ALU.min,
                    )

            # ---- fc4 + log_softmax ----
            # ltr[p, 32u+o] = logit class o of column 32u+p; column 32u+p
            # carries batch row 64p+u (host permutation), so partition p of
            # outf holds y rows 64p..64p+63 contiguously. Phase 1 per n
            # (through Exp/reduce) overlaps later fc4 matmuls; the Ln's
            # (different act table than Exp) run batched at the end.
            yb = y_t[:]
            b4r = b4bc[:]
            ltvs = []
            for n in range(NSPLIT):
                ps4 = psum.tile([OUT_F, NB], f32, tag="mm", name="ps4")
                for k in range(HC):
                    nc.tensor.matmul(
                        ps4, w4_sb[:, k], act3[:, k, ts(n, NB)],
                        start=(k == 0), stop=(k == HC - 1),
                    )
                nc.vector.tensor_scalar_add(
                    out=logits[0:OUT_F, ts(n, NB)], in0=ps4, scalar1=0.0
                )
                nc.vector.transpose(ltr[:, ts(n, NB)], logits[:, ts(n, NB)])
                base = ltr[:, ts(n, NB)]
                ltv = bass.AP(tensor=base.tensor, offset=base.offset,
                              ap=[base.ap[0], [32, NJ], [1, OUT_F]])
                ltvs.append(ltv)
                nc.vector.tensor_tensor(
                    out=ltv, in0=ltv,
                    in1=bass.AP(tensor=b4r.tensor, offset=b4r.offset,
                                ap=[[b4r.ap[0][0], 32], [0, NJ], b4r.ap[1]]),
                    op=ALU.add,
                )
                nc.scalar.activation(es[:, ts(n, NJ), :], ltv, AF.Exp)
                nc.vector.tensor_reduce(
                    out=lse[:, ts(n, NJ)], in_=es[:, ts(n, NJ), :],
                    axis=mybir.AxisListType.X, op=ALU.add,
                )
            nc.scalar.activation(lse, lse, AF.Ln)
            for n in range(NSPLIT):
                lsen = lse[:, ts(n, NJ)]
                nc.vector.tensor_tensor(
                    out=outf[:, ts(n, NJ), :], in0=ltvs[n],
                    in1=bass.AP(tensor=lsen.tensor, offset=lsen.offset,
                                ap=[lsen.ap[0], lsen.ap[1], [0, OUT_F]]),
                    op=ALU.subtract,
                )
                # y rows 64p + [16n, 16n+16) per partition p: contiguous 640B
                nc.sync.dma_start(
                    out=bass.AP(tensor=yb.tensor, offset=yb.offset + NJ * OUT_F * n,
                                ap=[[64 * OUT_F, 32], [OUT_F, NJ], [1, OUT_F]]),
                    in_=outf[:, ts(n, NJ), :],
                )

    nc.finalize()
    return nc


def _host_prep(inputs):
    """Shard x, binarize/lay out weights (partition-major), fold bn biases."""
    import ml_dtypes

    f16 = np.float16
    f8 = ml_dtypes.float8_e4m3

    x = np.asarray(inputs["x"], np.float32)
    w1 = np.asarray(inputs["w1"], np.float32)
    w2 = np.asarray(inputs["w2"], np.float32)
    w3 = np.asarray(inputs["w3"], np.float32)
    w4 = np.asarray(inputs["w4"], np.float32)
    b1 = np.asarray(inputs["b1"], np.float32)
    b2 = np.asarray(inputs["b2"], np.float32)
    b3 = np.asarray(inputs["b3"], np.float32)
    b4 = np.asarray(inputs["b4"], np.float32)

    EPS = np.float64(1e-5)

    def gv(i):
        return (np.asarray(inputs[f"g{i}"], np.float32),
                np.asarray(inputs[f"be{i}"], np.float32),
                np.asarray(inputs[f"m{i}"], np.float32),
                np.asarray(inputs[f"v{i}"], np.float32))

    g1, be1, m1, v1 = gv(1)
    g2, be2, m2, v2 = gv(2)
    g3, be3, m3, v3 = gv(3)
    # sign(bn(h)) == sign(h + (b - m)) requires gamma > 0 and beta == 0
    assert np.all(g1 > 0) and np.all(be1 == 0), "unsupported bn1 params"
    assert np.all(g2 > 0) and np.all(be2 == 0), "unsupported bn2 params"

    def pmaj(v):  # [1024] -> [128, 8] partition-major
        return np.ascontiguousarray(v.reshape(HC, 128).T)

    bias1 = pmaj(((b1 - m1) * 64.0).astype(np.float32))  # fc1 runs at 2^6
    bias2 = pmaj((b2 - m2).astype(np.float32))
    r3 = 1.0 / np.sqrt(v3.astype(np.float64) + EPS)
    sc3 = pmaj((r3 * g3).astype(np.float32))
    sh3 = pmaj(((b3 - m3).astype(np.float64) * r3 * g3 + be3).astype(np.float32))

    # fc1 weights: rows 0-767 split [p][m][k][c]; hi at sign*2^6 (f16),
    # lo at sign*2^-6 (fp8, min normal). Hi chunk 6 is the packed tail:
    # rows 768-783 replicated at p0-15 and p16-31 (hi/lo), zero elsewhere.
    s1f = np.sign(w1).T.astype(np.float32)              # [784, 1024]
    body = s1f[:768].reshape(6, 128, HC, 128).transpose(2, 0, 1, 3)  # [m,k,p,c]
    s1h = np.zeros((HC, KH, 128, 128), np.float32)
    s1h[:, :6] = body * 64.0
    tail = s1f[768:IN_F].reshape(16, HC, 128) * 64.0    # [16, 8, 128]
    for mm in range(HC):
        s1h[mm, 6, 0:16] = tail[:, mm]
        s1h[mm, 6, 16:32] = tail[:, mm]
    s1h = np.ascontiguousarray(s1h.transpose(2, 0, 1, 3)).astype(f16)  # [p,m,k,c]
    s1l = np.ascontiguousarray(
        (body * (2.0 ** -6)).transpose(2, 0, 1, 3)).astype(f8)         # [p,m,k,c]

    def wlay(w, dt):  # [out, in] -> [p, m, k, c] partition-major
        st = np.sign(w).T.astype(np.float32)            # [in, out]
        a = st.reshape(HC, 128, HC, 128).transpose(2, 0, 1, 3)  # [m,k,p,c]
        return np.ascontiguousarray(a.transpose(2, 0, 1, 3)).astype(dt)

    s2t = wlay(w2, f8)
    s3t = wlay(w3, f8)
    w4t = np.ascontiguousarray(
        w4.T.astype(f16).reshape(HC, 128, OUT_F).transpose(1, 0, 2))  # [p,m,o]

    shared = dict(s1h=s1h, s1l=s1l, s2t=s2t, s3t=s3t, w4t=w4t,
                  bias1=bias1, bias2=bias2, sc3=sc3, sh3=sh3, b4=b4)

    # column permutation: kernel column c carries batch row 64*(c%32)+c//32,
    # so the transposed output lands contiguously per partition.
    cc = np.arange(BS)
    perm = 64 * (cc % 32) + cc // 32

    in_maps = []
    for c in range(N_CORES):
        xs = x[c * BS:(c + 1) * BS][perm]               # [2048, 784] permuted
        xcols = np.ascontiguousarray(xs.T)              # [784, 2048] fp32
        xhi = xcols.astype(f16)
        lo32 = xcols - xhi.astype(np.float32)           # exact residual
        xh = np.zeros((KH, 128, BS), f16)
        xh[:6] = xhi[:768].reshape(6, 128, BS)
        xh[6, 0:16] = xhi[768:IN_F]
        xh[6, 16:32] = lo32[768:IN_F].astype(f16)
        # [k, p, (n nb)] -> [p, n, k, nb]
        xh = np.ascontiguousarray(
            xh.reshape(KH, 128, NSPLIT, NB).transpose(1, 2, 0, 3))
        xl = np.ascontiguousarray(
            (lo32[:768] * 4096.0).reshape(KL, 128, NSPLIT, NB)
            .transpose(1, 2, 0, 3)).astype(f8)
        m = dict(shared)
        m["xh"] = xh
        m["xl"] = xl
        in_maps.append(m)
    return in_maps


def kernel(**inputs):
    global LAST_RESULT
    from concourse.bass_utils import run_bass_kernel_spmd

    if "nc" not in _PLAN:
        _PLAN["nc"] = _build_nc()
    nc = _PLAN["nc"]

    in_maps = _host_prep(inputs)
    br = run_bass_kernel_spmd(
        nc, in_maps, list(range(N_CORES)),
        tmpdir=os.environ.get("KERNEL_TMPDIR") or None,
    )
    LAST_RESULT = br
    out = np.concatenate([br.results[c]["y"] for c in range(N_CORES)], axis=0)
    return out.astype(np.float32)
